# revision 1
# baseline (speedup 1.0000x reference)
"""Causal self-attention (RoPE) Trainium2 kernel, 8-way head-parallel.

Contract: kernel(**inputs) takes the full unsharded inputs
  x [B,T,C] f32, W_attn [C,3C] f32, W_proj [C,C] f32,
  rope_cos/rope_sin [T,D/2] f32, attention_mask [B,1,T,T] bool (all ones)
and returns the full output [B,T,C] f32.

Sharding (tensor-parallel over heads): core m computes q/k/v + attention
for heads {2m, 2m+1} of BOTH batches (no redundant work anywhere), then
one 1MB AllToAll per batch redistributes y token-wise so each core runs
the output projection for tokens [256m, 256m+256) of each batch.

Schedule: q/k/v live in per-(head-unit, token-quarter) tiles, so each
attention chunk (unit u, query block qb) is emitted as soon as the
quarters it reads exist. QKV projection of both batches interleaves
with attention chunks — the scalar-engine exp stream hides under QKV
matmuls instead of pacing a long attention tail. Batch-0's AllToAll
flies while late attention runs; batch-0's projection overlaps batch-1's
AllToAll. Attention exploits causal structure (query block qb attends
key blocks 0..4qb+3; only diagonal blocks are masked); the softmax
denominator accumulates in PSUM via ones-matmuls (gpsimd stays free so
collectives dispatch the moment their packs land).

All matmul operands bf16; weights/x pre-arranged on host so every SBUF
load is one contiguous DMA.
"""

import sys

sys.path.insert(0, "/opt/trn_rl_repo")

import numpy as np

import concourse.bacc as bacc
import concourse.bass as bass
import concourse.mybir as mybir
import concourse.tile as tile
from concourse.bass_utils import run_bass_kernel_spmd

F32 = mybir.dt.float32
BF16 = mybir.dt.bfloat16
MM_DT = BF16
N_CORES = 8
B, T, C = 2, 2048, 2048
D = 128
H = C // D            # 16 heads
HPC = H // N_CORES    # 2 heads per core
NU = B * HPC          # 4 head-units per core (2 heads x 2 batches)
NCC = C // 128        # 16 contraction chunks
NQB = T // 512        # 4 query blocks (= token quarters) per unit
NKB = T // 128        # 16 key blocks per unit
TOK = T // N_CORES    # 256 output tokens per core per batch

# emission schedule: A = qkv quarter (batch, quarter), B = attention
# chunk (unit, query block), REL = release phase-A pools, PK = pack y
SCHED = [
    ("A", 0, 0), ("A", 0, 1), ("A", 0, 2), ("A", 0, 3),
    ("A", 1, 0),
    ("B", 0, 3), ("B", 0, 2),
    ("A", 1, 1),
    ("B", 1, 3), ("B", 0, 1), ("B", 0, 0), ("PK", 0),
    ("A", 1, 2),
    ("B", 1, 2), ("B", 1, 1), ("B", 1, 0), ("PK", 1),
    ("A", 1, 3),
    ("REL",),
    ("B", 2, 3), ("B", 2, 2), ("B", 2, 1), ("B", 2, 0), ("PK", 2),
    ("B", 3, 3), ("B", 3, 2), ("B", 3, 1), ("B", 3, 0), ("PK", 3),
]


def build_nc(repeat=1):
    nc = bacc.Bacc(None)
    xq = nc.dram_tensor("xq", [B * 4, 128, NCC, 512], MM_DT, kind="ExternalInput")
    Wq = nc.dram_tensor("Wq", [128, HPC, NCC, 128], MM_DT, kind="ExternalInput")
    Wk = nc.dram_tensor("Wk", [128, HPC, NCC, 128], MM_DT, kind="ExternalInput")
    Wv = nc.dram_tensor("Wv", [128, NCC, HPC * 128], MM_DT, kind="ExternalInput")
    Wp = nc.dram_tensor("Wp", [128, NCC, 4, 512], MM_DT, kind="ExternalInput")
    cc = nc.dram_tensor("cc", [128, T], MM_DT, kind="ExternalInput")
    ss = nc.dram_tensor("ss", [128, T], MM_DT, kind="ExternalInput")
    dmask = nc.dram_tensor("dmask", [128, 4, 512], MM_DT, kind="ExternalInput")
    ones = nc.dram_tensor("ones", [128, 128], MM_DT, kind="ExternalInput")
    out_d = nc.dram_tensor("out", [B, TOK, C], F32, kind="ExternalOutput")

    scale = 1.0 / float(np.sqrt(np.float32(D)))
    EXP = mybir.ActivationFunctionType.Exp

    with tile.TileContext(nc) as tc:
        from contextlib import ExitStack
        es_all = ExitStack()
        with es_all:
            ec = es_all.enter_context
            p_dram = ec(tc.tile_pool(name="dram", bufs=1, space="DRAM"))
            a2a_in = [p_dram.tile([N_CORES, 128, TOK], MM_DT,
                                  tag=f"a2a_in{u}", name=f"a2a_in{u}")
                      for u in range(NU)]
            a2a_out = [p_dram.tile([N_CORES, 128, TOK], MM_DT,
                                   tag=f"a2a_out{u}", name=f"a2a_out{u}")
                       for u in range(NU)]
            # persistent left-side pools
            p_wp = ec(tc.tile_pool(name="wp", bufs=1))
            p_qkv = ec(tc.tile_pool(name="qkv", bufs=1))
            p_y = ec(tc.tile_pool(name="y", bufs=1))
            p_es = ec(tc.tile_pool(name="es", bufs=6))
            p_ea = ec(tc.tile_pool(name="ea", bufs=1))
            p_rc = ec(tc.tile_pool(name="rc", bufs=1))
            p_msk = ec(tc.tile_pool(name="msk", bufs=1))
            # psum
            p_mm = ec(tc.tile_pool(name="psMM", bufs=4, space="PSUM"))
            p_vp = ec(tc.tile_pool(name="psV", bufs=1, space="PSUM"))
            p_acc = ec(tc.tile_pool(name="psAcc", bufs=3, space="PSUM"))

            for rep in range(repeat):
                # phase-A pools: right-side stack, released mid-rep (space
                # reused by the ya/ost pools of phase C; LIFO per side)
                p_w = tc.alloc_tile_pool(name="w", bufs=1, side="right")
                p_ct = tc.alloc_tile_pool(name="ct", bufs=2, side="right")
                p_x = tc.alloc_tile_pool(name="xt", bufs=2, side="right")
                p_rope = tc.alloc_tile_pool(name="rope", bufs=2, side="right")

                wq_sb = p_w.tile([128, HPC, NCC, 128], MM_DT, tag="wq")
                wk_sb = p_w.tile([128, HPC, NCC, 128], MM_DT, tag="wk")
                wv_sb = p_w.tile([128, NCC, HPC * 128], MM_DT, tag="wv")
                nc.scalar.dma_start(wq_sb[:, 0, 0, :], Wq[:, 0, 0, :])
                nc.scalar.dma_start(wq_sb[:, 0, 1:, :], Wq[:, 0, 1:, :])
                nc.scalar.dma_start(wq_sb[:, 1, :, :], Wq[:, 1, :, :])
                nc.gpsimd.dma_start(wk_sb[:], Wk[:])
                nc.gpsimd.dma_start(wv_sb[:], Wv[:])
                dm_sb = p_msk.tile([128, 4, 512], MM_DT, tag="dm")
                nc.scalar.dma_start(dm_sb[:], dmask[:])
                ones_sb = p_msk.tile([128, 128], MM_DT, tag="ones")
                nc.scalar.dma_start(ones_sb[:], ones[:])
                # Wp is a resident tile; its 8MB load hides in early idle
                wp_sb = p_wp.tile([128, NCC, 4, 512], MM_DT, tag="wp")
                nc.gpsimd.dma_start(wp_sb[:], Wp[:])

                # per-(unit, quarter) tiles
                qT = [[p_qkv.tile([128, 512], MM_DT, tag=f"qT{u}_{t}",
                                  name=f"qT{u}_{t}") for t in range(4)]
                      for u in range(NU)]
                kT = [[p_qkv.tile([128, 512], MM_DT, tag=f"kT{u}_{t}",
                                  name=f"kT{u}_{t}") for t in range(4)]
                      for u in range(NU)]
                vv = [[p_qkv.tile([128, 4, 128], MM_DT, tag=f"v{u}_{t}",
                                  name=f"v{u}_{t}") for t in range(4)]
                      for u in range(NU)]
                yy = [p_y.tile([128, T], MM_DT, tag=f"y{u}", name=f"y{u}")
                      for u in range(NU)]

                def rope(dst_ap, src_ps, ct, st):
                    """dst = src*cos + swap64(src)*(-/+sin); src in PSUM."""
                    sw = p_rope.tile([128, 512], MM_DT, tag="rp_sw")
                    nc.scalar.copy(sw[0:64, :], src_ps[64:128, :])
                    nc.scalar.copy(sw[64:128, :], src_ps[0:64, :])
                    t1 = p_rope.tile([128, 512], MM_DT, tag="rp_t1")
                    nc.vector.tensor_mul(t1[:], src_ps[:], ct[:])
                    nc.vector.tensor_mul(sw[:], sw[:], st[:])
                    nc.vector.tensor_add(dst_ap, t1[:], sw[:])

                def emit_A_quarter(b, tt):
                    """qkv projection + rope for tokens [tt*512,(tt+1)*512)."""
                    xt = p_x.tile([128, NCC, 512], MM_DT, tag="xt")
                    if b == 0 and tt == 0:
                        # sync carries the first 11 chunks; scalar takes the
                        # tail after the rope tables (emitted below)
                        nc.sync.dma_start(xt[:, 0:11], xq[0][:, 0:11])
                    else:
                        nc.sync.dma_start(xt[:], xq[b * 4 + tt])
                    ct = p_ct.tile([128, 512], MM_DT, tag="ct")
                    st = p_ct.tile([128, 512], MM_DT, tag="st")
                    nc.scalar.dma_start(ct[:], cc[:, tt * 512:(tt + 1) * 512])
                    nc.scalar.dma_start(st[:], ss[:, tt * 512:(tt + 1) * 512])
                    if b == 0 and tt == 0:
                        nc.scalar.dma_start(xt[:, 11:], xq[0][:, 11:])
                    for h in range(HPC):
                        for wsb, dst in ((wq_sb, qT), (wk_sb, kT)):
                            ps = p_mm.tile([128, 512], F32, tag="ps")
                            for c in range(NCC):
                                nc.tensor.matmul(
                                    ps[:], wsb[:, h, c, :], xt[:, c, :],
                                    start=(c == 0), stop=(c == NCC - 1),
                                )
                            rope(dst[b * HPC + h][tt][:], ps, ct, st)
                    for r in range(4):
                        vp = p_vp.tile([128, HPC * 128], F32, tag="vp")
                        for c in range(NCC):
                            nc.tensor.matmul(
                                vp[:], xt[:, c, r * 128:(r + 1) * 128],
                                wv_sb[:, c, :],
                                start=(c == 0), stop=(c == NCC - 1),
                            )
                        for h in range(HPC):
                            nc.vector.tensor_copy(
                                vv[b * HPC + h][tt][:, r, :],
                                vp[:, h * 128:(h + 1) * 128],
                            )

                def emit_B_chunk(u, qb):
                    """attention for query block qb of head-unit u.

                    Softmax denominator: big blocks (qb>=2) accumulate exp
                    sums on DVE (their serial add-chain is still shorter
                    than the scalar exp stream) + one ones-matmul; small
                    blocks keep per-kb PE ones-matmuls (cheap, and the DVE
                    chain overhead isn't worth it).
                    """
                    nkbv = qb * 4 + 4
                    on_dve = u < 2 and qb >= 2
                    yps = p_acc.tile([128, 512], F32, tag="acc")
                    if on_dve:
                        ea = p_ea.tile([128, 512], F32, tag="ea")
                    else:
                        csps = p_acc.tile([128, 512], F32, tag="acc")
                    for kb in range(nkbv):
                        tt, r = kb // 4, kb % 4
                        sps = p_mm.tile([128, 512], F32, tag="ps")
                        nc.tensor.matmul(
                            sps[:], kT[u][tt][:, r * 128:(r + 1) * 128],
                            qT[u][qb][:],
                            start=True, stop=True,
                        )
                        es = p_es.tile([128, 512], MM_DT, tag="es")
                        nc.scalar.activation(es[:], sps[:], EXP, scale=scale)
                        if kb >= qb * 4:
                            nc.vector.tensor_mul(
                                es[:], es[:], dm_sb[:, kb - qb * 4, :])
                        if on_dve:
                            if kb == 0:
                                nc.vector.tensor_copy(ea[:], es[:])
                            else:
                                nc.vector.tensor_add(ea[:], ea[:], es[:])
                        else:
                            nc.tensor.matmul(
                                csps[:], ones_sb[:], es[:],
                                start=(kb == 0), stop=(kb == nkbv - 1),
                            )
                        nc.tensor.matmul(
                            yps[:], vv[u][tt][:, r, :], es[:],
                            start=(kb == 0), stop=(kb == nkbv - 1),
                        )
                    if on_dve:
                        eab = p_rc.tile([128, 512], MM_DT, tag="eab")
                        nc.vector.tensor_copy(eab[:], ea[:])
                        csps = p_acc.tile([128, 512], F32, tag="acc")
                        nc.tensor.matmul(csps[:], ones_sb[:], eab[:],
                                         start=True, stop=True)
                    rc = p_rc.tile([128, 512], F32, tag="rc")
                    nc.vector.reciprocal(rc[:], csps[:])
                    nc.vector.tensor_mul(
                        yy[u][:, qb * 512:(qb + 1) * 512], yps[:], rc[:])

                def emit_pack(u):
                    nc.gpsimd.dma_start(
                        a2a_in[u].rearrange("j p x -> p j x"),
                        yy[u].rearrange("p (j x) -> p j x", x=TOK),
                    )

                for op in SCHED:
                    if op[0] == "A":
                        emit_A_quarter(op[1], op[2])
                    elif op[0] == "B":
                        emit_B_chunk(op[1], op[2])
                    elif op[0] == "PK":
                        emit_pack(op[1])
                    elif op[0] == "REL":
                        for p in (p_rope, p_x, p_ct, p_w):
                            p.release()

                # per-unit AllToAlls: each dispatches as soon as its pack
                # lands (gpsimd queue is clear); only the last is exposed.
                def emit_cc(u):
                    nc.gpsimd.collective_compute(
                        "AllToAll", mybir.AluOpType.bypass,
                        replica_groups=[list(range(N_CORES))],
                        ins=[a2a_in[u].opt()],
                        outs=[a2a_out[u].opt()],
                    )

                emit_cc(0)
                emit_cc(1)

                p_ya = tc.alloc_tile_pool(name="ya", bufs=1, side="right")
                p_ost = tc.alloc_tile_pool(name="ost", bufs=8, side="right")

                def emit_proj(b, ya, defer_stores=False):
                    for h in range(HPC):
                        nc.sync.dma_start(
                            ya[:, h::HPC, :],
                            a2a_out[b * HPC + h].rearrange("i p x -> p i x"),
                        )
                    stores = []
                    for t in range(TOK // 128):
                        pss = [
                            p_mm.tile([128, 512], F32, tag="ps",
                                      name=f"ops{b}{t}{g}")
                            for g in range(3)
                        ] + [p_acc.tile([128, 512], F32, tag="acc",
                                        name=f"ops{b}{t}3")]
                        for hc in range(NCC):
                            for g in range(4):
                                nc.tensor.matmul(
                                    pss[g][:],
                                    ya[:, hc, t * 128:(t + 1) * 128],
                                    wp_sb[:, hc, g, :],
                                    start=(hc == 0), stop=(hc == NCC - 1),
                                )
                        for g in range(4):
                            ost = p_ost.tile([128, 512], F32, tag="ost")
                            nc.scalar.copy(ost[:], pss[g][:])
                            dst = out_d[b, t * 128:(t + 1) * 128,
                                        g * 512:(g + 1) * 512]
                            if defer_stores:
                                stores.append((dst, ost))
                            else:
                                nc.sync.dma_start(dst, ost[:])
                    return stores

                # proj(b0) overlaps cc_u2/cc_u3 (emitted after it); b0
                # stores deferred past those collectives' emission.
                ya0 = p_ya.tile([128, NCC, TOK], MM_DT, tag="ya0", name="ya0")
                st0 = emit_proj(0, ya0, defer_stores=True)
                emit_cc(2)
                # proj(b1) first half: u2's (even) channel chunks arrive with
                # cc_u2, so this half of the contraction hides under cc_u3.
                ya1 = p_ya.tile([128, NCC, TOK], MM_DT, tag="ya1", name="ya1")
                nc.sync.dma_start(
                    ya1[:, 0::HPC, :],
                    a2a_out[2].rearrange("i p x -> p i x"),
                )
                part = []
                for t in range(TOK // 128):
                    pss = [
                        p_mm.tile([128, 512], F32, tag="ps",
                                  name=f"pb1a{t}{g}")
                        for g in range(3)
                    ] + [p_acc.tile([128, 512], F32, tag="acc",
                                    name=f"pb1a{t}3")]
                    for hc in range(0, NCC, HPC):
                        for g in range(4):
                            nc.tensor.matmul(
                                pss[g][:],
                                ya1[:, hc, t * 128:(t + 1) * 128],
                                wp_sb[:, hc, g, :],
                                start=(hc == 0), stop=(hc == NCC - HPC),
                            )
                    for g in range(4):
                        pt = p_ost.tile([128, 512], F32, tag="part",
                                        name=f"part{t}{g}")
                        nc.scalar.copy(pt[:], pss[g][:])
                        part.append(pt)
                emit_cc(3)
                for dst, ost in st0:
                    nc.sync.dma_start(dst, ost[:])
                # second half: u3's (odd) channel chunks + merge + store
                nc.sync.dma_start(
                    ya1[:, 1::HPC, :],
                    a2a_out[3].rearrange("i p x -> p i x"),
                )
                for t in range(TOK // 128):
                    pss = [
                        p_mm.tile([128, 512], F32, tag="ps",
                                  name=f"pb1b{t}{g}")
                        for g in range(3)
                    ] + [p_acc.tile([128, 512], F32, tag="acc",
                                    name=f"pb1b{t}3")]
                    # g-outer: each group's stop lands early so its merge-add
                    # and store overlap the later groups' matmuls
                    for g in range(4):
                        for hc in range(1, NCC, HPC):
                            nc.tensor.matmul(
                                pss[g][:],
                                ya1[:, hc, t * 128:(t + 1) * 128],
                                wp_sb[:, hc, g, :],
                                start=(hc == 1), stop=(hc == NCC - 1),
                            )
                        ost = p_ost.tile([128, 512], F32, tag="ost")
                        nc.vector.tensor_add(ost[:], pss[g][:],
                                             part[t * 4 + g][:])
                        eng = (nc.sync, nc.scalar, nc.gpsimd, nc.sync)[g]
                        eng.dma_start(
                            out_d[1, t * 128:(t + 1) * 128,
                                  g * 512:(g + 1) * 512],
                            ost[:])
                for p in (p_ost, p_ya):
                    p.release()

    nc.compile()
    return nc


def _prep_inputs(x, W_attn, W_proj, rope_cos, rope_sin):
    """Host-side prep. Returns in_maps for the 8 cores."""
    import ml_dtypes
    bf = ml_dtypes.bfloat16

    x = np.asarray(x, dtype=np.float32)
    W_attn = np.asarray(W_attn, dtype=np.float32)
    W_proj = np.asarray(W_proj, dtype=np.float32)
    rope_cos = np.asarray(rope_cos, dtype=np.float32)
    rope_sin = np.asarray(rope_sin, dtype=np.float32)

    xq = (x.reshape(B, 4, 512, NCC, 128).transpose(0, 1, 4, 3, 2)
          .astype(bf).reshape(B * 4, 128, NCC, 512))

    # per-head rope permutation of q/k columns: (evens | odds)
    perm = np.concatenate([np.arange(0, D, 2), np.arange(1, D, 2)])
    colperm = np.concatenate([h * D + perm for h in range(H)])

    def qk_tile(w):  # [C, C] -> [128, H, ncc, 128]
        return (w[:, colperm].reshape(NCC, 128, H, 128)
                .transpose(1, 2, 0, 3).astype(bf))

    Wq_t = qk_tile(W_attn[:, 0:C])
    Wk_t = qk_tile(W_attn[:, C:2 * C])
    Wv_t = (W_attn[:, 2 * C:3 * C].reshape(NCC, 128, H, 128)
            .transpose(1, 0, 2, 3).astype(bf))          # [128, ncc, H, 128]
    Wp_t = (W_proj.reshape(NCC, 128, 4, 512)
            .transpose(1, 0, 2, 3).astype(bf))          # [128, hc, ocg, 512]

    # rope tables in permuted layout: rows 0:64 real-pair, 64:128 imag-pair
    cosT = rope_cos.T.astype(np.float32)  # [64, T]
    sinT = rope_sin.T.astype(np.float32)
    cc_t = np.concatenate([cosT, cosT], axis=0).astype(bf)
    ss_t = np.concatenate([-sinT, sinT], axis=0).astype(bf)

    # diagonal masks: dm[p, r, q] = (r*128 + p) <= q
    dm = ((np.arange(4)[None, :, None] * 128 + np.arange(128)[:, None, None]
           <= np.arange(512)[None, None, :]).astype(bf))
    ones_t = np.ones((128, 128), dtype=bf)

    in_maps = []
    for m in range(N_CORES):
        hsl = slice(HPC * m, HPC * (m + 1))
        in_maps.append({
            "xq": xq, "cc": cc_t, "ss": ss_t, "dmask": dm, "ones": ones_t,
            "Wp": Wp_t,
            "Wq": np.ascontiguousarray(Wq_t[:, hsl]),
            "Wk": np.ascontiguousarray(Wk_t[:, hsl]),
            "Wv": np.ascontiguousarray(Wv_t[:, :, hsl, :]).reshape(
                128, NCC, HPC * 128),
        })
    return in_maps


_NC_CACHE = {}


def run(x, W_attn, W_proj, rope_cos, rope_sin, attention_mask=None):
    if "nc" not in _NC_CACHE:
        _NC_CACHE["nc"] = build_nc()
    nc = _NC_CACHE["nc"]
    in_maps = _prep_inputs(x, W_attn, W_proj, rope_cos, rope_sin)
    res = run_bass_kernel_spmd(nc, in_maps, list(range(N_CORES)))
    out = np.empty((B, T, C), dtype=np.float32)
    for m in range(N_CORES):
        out[:, m * TOK:(m + 1) * TOK, :] = res.results[m]["out"]
    return out, res


def kernel(x, W_attn, W_proj, rope_cos, rope_sin, attention_mask):
    out, _ = run(x, W_attn, W_proj, rope_cos, rope_sin)
    return out

